# revision 7
# baseline (speedup 1.0000x reference)
"""Trainium2 Bass kernel for nn_AggregateStgcn (gnn_message_passing).

Computes, for x:(1,16,1,8192) f32, graph:(8192,8192) f32, fifo:(1,16,4,8192) f32,
stride=2:
    Asum[k, v] = sum_c x[0, c*4+k, 0, v]              (4, 8192)
    xsum[k, w] = sum_v Asum[k, v] * graph[v, w]       (4, 8192)
    S[k, w]    = sum_{j in 1,3,...,13} fifo[0, j, k, w]
    out[0, k, w, 0] = xsum[k, w] + S[k, w]            (1, 4, 8192, 1)

Sharding: graph is split column-wise across 8 NeuronCores (tensor parallel over
output nodes w); x is replicated; the fifo slice is local per core. No
collectives; host concatenates the 8 (4, 1024) output slices.

Precision/perf strategy: the kernel streams the whole graph once, so it is
HBM-bandwidth-bound; the graph is quantized on the host to 1 byte/element.
Rows are split in two regimes to balance the PE against the DMA:
 - tiles 0..NSPLIT-1: fp8 E3M4 (4 mantissa bits) at scale 256, matmul'd
   against the bf16 activation at 1 moving column/cycle;
 - tiles NSPLIT..63: fp8 E4M3 at scale 1024 in DoubleRow perf mode (2 graph
   elements/cycle, halving PE time for these rows). DoubleRow requires an
   fp8 stationary operand, so the activation is packed as E4M3 hi (cols 0:4,
   scale 1/8) plus E4M3 lo of the residual (cols 32:36, 16x finer scale) -
   the pair recovers ~bf16 activation accuracy. These rows accumulate in a
   separate (48, 512) PSUM pair folded in at the end as
   out += 2^-7 * acc_hi + 2^-11 * acc_lo.
Measured end-to-end error of this scheme on the real inputs: 1.27e-2
(max-err / max-expected) vs the 2e-2 gate; the HW matmul paths are exact on
quantized values (verified against a host simulation to ~1e-4).

Layout: the host pre-permutes the graph slice into the exact per-partition
stream order (for a chunk of s row-tiles starting at row off*128, partition p
holds rows off*128 + p*s + j, j=0..s-1), so every DMA is a plain 2D slice
with one long contiguous run per partition. x is shipped pre-transposed and
identically permuted as (128, 64*16) bf16 (pre-divided by 256); three DVE
adds reduce its 16 channels to the (128, 64, 4) bf16 stationary operand.
In a DoubleRow pair the two row-tiles are the k2=0/1 halves (contraction
index = partition + 128*k2), which is exactly two consecutive stream tiles.

Schedule: tiny chunks first (the first matmuls start ~2 us after the DMA
ramp), 512 KB chunks in the middle alternating across the two HWDGE queues
(keeps the arrival cadence under the PE's consumption rate), DoubleRow
chunks last (they drain 2x faster after the final bytes land). Throwaway
matmuls open the PE HAM clock gate during the ramp, and filler matmuls after
the early chunks bridge arrival gaps so the gate never drops to half clock.
"""

import numpy as np

V = 8192
C = 4
K = 4
F = 16
NCORES = 8
WS = V // NCORES          # 1024 output columns per core
NT = V // 128             # 64 contraction tiles
NSPLIT = 40               # tiles 0..39 e3m4 normal, 40..63 e4m3 DoubleRow
# v-tiles per DMA chunk (normal region, then DoubleRow region)
CHUNKS_N = [1, 1, 2, 2] + [4] * 8 + [2]
CHUNKS_D = [4, 4, 4, 4, 4, 2, 2]
assert sum(CHUNKS_N) == NSPLIT and sum(CHUNKS_D) == NT - NSPLIT
CHUNKS = CHUNKS_N + CHUNKS_D
GBUFS = 10                # graph chunk buffers in SBUF
WARMUP_MM = 7             # throwaway matmuls to open the PE clock gate
FILLER_CHUNKS = 6         # early chunks followed by 2 filler matmuls each
GSCALE = 256.0            # e3m4 graph pre-scale (x is pre-divided by this)
G4SCALE = 1024.0          # e4m3 graph pre-scale for the DoubleRow region
# DoubleRow stationary: asum*32 = A/8 in e4m3 hi, residual*16 in e4m3 lo;
# products come out 2^7 (hi) and 2^11 (lo) too large
FOLD_HI = 2.0 ** -7
FOLD_LO = 2.0 ** -11
DRW = 48                  # DoubleRow stationary padded width (16B-aligned)

TRACE = False             # set by test harness to capture an NTFF profile
LAST = None               # BassKernelResults of the most recent run

_CACHED_NC = None


def _offs():
    return np.cumsum([0] + CHUNKS).tolist()


def _vmap():
    """vmap[t, p] = graph row held by partition p for contraction tile t."""
    offs = _offs()
    vm = np.empty((NT, 128), np.int64)
    for ci, s in enumerate(CHUNKS):
        off = offs[ci]
        for j in range(s):
            vm[off + j] = off * 128 + np.arange(128) * s + j
    return vm


def _build_nc():
    import concourse.bacc as bacc
    import concourse.mybir as mybir
    from concourse.tile import TileContext

    f32 = mybir.dt.float32
    bf16 = mybir.dt.bfloat16
    f8e3 = mybir.dt.float8e3
    f8e4 = mybir.dt.float8e4
    nc = bacc.Bacc(
        "TRN2",
        target_bir_lowering=False,
        debug=False,
        enable_asserts=False,
        num_devices=NCORES,
    )
    NDR = NT - NSPLIT
    g8 = nc.dram_tensor("g8", [128, NSPLIT * WS], f8e3, kind="ExternalInput")
    g4 = nc.dram_tensor("g4", [128, NDR * WS], f8e4, kind="ExternalInput")
    xtd = nc.dram_tensor("xtd", [128, NT * C * K], bf16, kind="ExternalInput")
    ffhi = nc.dram_tensor("ffhi", [7 * C, WS], bf16, kind="ExternalInput")
    selfm = nc.dram_tensor("selfm", [7 * C, K], bf16, kind="ExternalInput")
    out = nc.dram_tensor("out", [K, WS], f32, kind="ExternalOutput")

    offs = _offs()
    n_chunks = len(CHUNKS)

    with TileContext(nc) as tc:
        with (
            tc.tile_pool(name="const", bufs=1) as cpool,
            tc.tile_pool(name="gp", bufs=GBUFS) as gpool,
            tc.tile_pool(name="ps", bufs=1, space="PSUM") as ppool,
        ):
            # PE warmup: throwaway bf16 matmuls with no input dependencies
            # beyond a memset, so the clock gate opens while data streams in.
            wtile = cpool.tile([128, 512], bf16)
            nc.vector.memset(wtile[:], 1.0)
            wps = ppool.tile([128, 512], f32)
            for _ in range(WARMUP_MM):
                nc.tensor.matmul(
                    wps[:], wtile[:, 0:128], wtile[:], start=True, stop=True
                )

            # small inputs first on both HWDGE queues, ahead of the graph
            xtd_sb = cpool.tile([128, NT * C * K], bf16)
            nc.sync.dma_start(out=xtd_sb[:], in_=xtd.ap())
            selfm_sb = cpool.tile([7 * C, K], bf16)
            nc.scalar.dma_start(out=selfm_sb[:], in_=selfm.ap())
            ffhi_sb = cpool.tile([7 * C, WS], bf16)
            nc.scalar.dma_start(out=ffhi_sb[:], in_=ffhi.ap())

            # graph chunk DMAs: queue them all up front, alternating queues
            gts = []
            for ci, s in enumerate(CHUNKS):
                off = offs[ci]
                if ci < len(CHUNKS_N):
                    gt = gpool.tile([128, s * WS], f8e3, name="gt", tag="gt")
                    src = g8.ap()[:, off * WS : (off + s) * WS]
                else:
                    doff = off - NSPLIT
                    gt = gpool.tile([128, s * WS], f8e4, name="gt4", tag="gt4")
                    src = g4.ap()[:, doff * WS : (doff + s) * WS]
                if ci % 2 == 0:
                    nc.sync.dma_start(out=gt[:], in_=src)
                else:
                    nc.scalar.dma_start(out=gt[:], in_=src)
                gts.append(gt)

            # DVE prep: reduce the 16 channels of the pre-transposed x to the
            # (128, 64, 4) stationary operand (bf16 in/out for 2x DVE rate;
            # the intermediate roundings are far below the fp8 graph error)
            xv = xtd_sb.rearrange("p (t a) -> p t a", a=C * K)
            t0 = cpool.tile([128, NT, K], bf16)
            nc.vector.tensor_add(out=t0[:], in0=xv[:, :, 0:K], in1=xv[:, :, K : 2 * K])
            t1 = cpool.tile([128, NT, K], bf16)
            nc.vector.tensor_add(
                out=t1[:], in0=xv[:, :, 2 * K : 3 * K], in1=xv[:, :, 3 * K :]
            )
            asum = cpool.tile([128, NT, K], bf16)
            nc.vector.tensor_add(out=asum[:], in0=t0[:], in1=t1[:])

            # DoubleRow stationary prep: asum_dr[p, t', 0:4] = e4m3(asum*32),
            # asum_dr[p, t', 32:36] = e4m3((asum*32 - hi) * 16), zeros between
            asum_dr = cpool.tile([128, NDR, DRW], f8e4)
            nc.vector.memset(asum_dr[:], 0.0)
            a32 = cpool.tile([128, NDR, K], f32)
            nc.vector.tensor_scalar_mul(a32[:], asum[:, NSPLIT:, :], 32.0)
            nc.vector.tensor_copy(out=asum_dr[:, :, 0:K], in_=a32[:])
            resid = cpool.tile([128, NDR, K], f32)
            nc.vector.tensor_sub(
                out=resid[:], in0=a32[:], in1=asum_dr[:, :, 0:K]
            )
            nc.vector.tensor_scalar_mul(asum_dr[:, :, 32:36], resid[:], 16.0)

            # accumulators: (4, 1024) main spanning two PSUM banks, plus a
            # (48, 512) pair for the DoubleRow region (hi rows 0:4, lo rows
            # 32:36). The fifo matmul opens the main accumulation group.
            accT = ppool.tile([K, WS], f32, name="acc", tag="acc")
            acc = [accT[:, 0:512], accT[:, 512:1024]]
            accd = [
                ppool.tile([DRW, 512], f32, name=f"accd{h}", tag=f"accd{h}")
                for h in range(2)
            ]
            for h in range(2):
                hs = slice(h * 512, (h + 1) * 512)
                nc.tensor.matmul(
                    acc[h], selfm_sb[:], ffhi_sb[:, hs], start=True, stop=False
                )

            for ci, s in enumerate(CHUNKS_N):
                off = offs[ci]
                gt = gts[ci]
                for j in range(s):
                    t = off + j
                    last = t == NSPLIT - 1
                    lhsT = asum[:, t, :]
                    for h in range(2):
                        hs = slice(j * WS + h * 512, j * WS + (h + 1) * 512)
                        nc.tensor.matmul(
                            acc[h], lhsT, gt[:, hs], start=False, stop=last,
                        )
                if 1 <= ci <= FILLER_CHUNKS:
                    # bridge early arrival gaps so the HAM clock gate never
                    # sees an idle window and drops the PE to half clock
                    for _ in range(2):
                        nc.tensor.matmul(
                            wps[:], wtile[:, 0:128], wtile[:],
                            start=True, stop=True,
                        )

            dr = mybir.MatmulPerfMode.DoubleRow
            for ci in range(len(CHUNKS_N), n_chunks):
                s = CHUNKS[ci]
                off = offs[ci]
                gt = gts[ci]
                gtv = gt.rearrange("p (j w) -> p j w", w=WS)
                for d in range(s // 2):
                    gdt = off - NSPLIT + 2 * d      # stream double-pair base
                    first = gdt == 0
                    lastd = gdt == NDR - 2
                    lhsT = asum_dr[:, gdt : gdt + 2, :]
                    for h in range(2):
                        nc.tensor.matmul(
                            accd[h][:],
                            lhsT,
                            gtv[:, 2 * d : 2 * d + 2, h * 512 : (h + 1) * 512],
                            start=first,
                            stop=lastd,
                            perf_mode=dr,
                        )

            # fold: out = accT + 2^-7 * accd[0:4] + 2^-11 * accd[32:36];
            # ACT does the scaled PSUM reads, DVE the adds, per half
            out_sb = cpool.tile([K, WS], f32)
            copy_fn = mybir.ActivationFunctionType.Copy
            for h in range(2):
                hs = slice(h * 512, (h + 1) * 512)
                fhi = cpool.tile([K, 512], f32, name=f"fhi{h}", tag=f"fhi{h}")
                nc.scalar.activation(fhi[:], accd[h][0:K, :], copy_fn, scale=FOLD_HI)
                flo = cpool.tile([K, 512], f32, name=f"flo{h}", tag=f"flo{h}")
                nc.scalar.activation(flo[:], accd[h][32:36, :], copy_fn, scale=FOLD_LO)
                fs = cpool.tile([K, 512], f32, name=f"fs{h}", tag=f"fs{h}")
                nc.vector.tensor_add(out=fs[:], in0=fhi[:], in1=flo[:])
                nc.vector.tensor_add(out=out_sb[:, hs], in0=fs[:], in1=acc[h])
                if h == 0:
                    nc.sync.dma_start(out=out.ap()[:, 0:512], in_=out_sb[:, 0:512])
                else:
                    nc.scalar.dma_start(
                        out=out.ap()[:, 512:1024], in_=out_sb[:, 512:1024]
                    )

    nc.compile()
    return nc


def kernel(x, graph, fifo, stride):
    global _CACHED_NC, LAST
    import ml_dtypes
    from concourse.bass_utils import run_bass_kernel_spmd

    bf16 = ml_dtypes.bfloat16
    e3m4 = ml_dtypes.float8_e3m4
    e4m3 = ml_dtypes.float8_e4m3
    x = np.asarray(x, dtype=np.float32)
    graph = np.asarray(graph, dtype=np.float32)
    fifo = np.asarray(fifo, dtype=np.float32)
    stride_v = int(np.asarray(stride))
    assert stride_v == 2, f"kernel hardcodes stride=2, got {stride_v}"

    vm = _vmap()                                  # (NT, 128)

    # graph rows permuted into stream order; e3m4 region then e4m3 region
    rows_n = np.ascontiguousarray(vm[:NSPLIT].T).reshape(-1)
    rows_d = np.ascontiguousarray(vm[NSPLIT:].T).reshape(-1)
    gq3 = np.clip(graph[rows_n] * GSCALE, -15.5, 15.5).astype(e3m4)
    gq4 = np.clip(graph[rows_d] * G4SCALE, -240.0, 240.0).astype(e4m3)
    g3v = gq3.reshape(128, NSPLIT, NCORES, WS)
    g4v = gq4.reshape(128, NT - NSPLIT, NCORES, WS)
    g8_sh = [
        np.ascontiguousarray(g3v[:, :, m]).reshape(128, NSPLIT * WS)
        for m in range(NCORES)
    ]
    g4_sh = [
        np.ascontiguousarray(g4v[:, :, m]).reshape(128, (NT - NSPLIT) * WS)
        for m in range(NCORES)
    ]

    # x -> (128, NT*16) bf16, transposed + identically permuted, pre-divided
    # by the e3m4 graph scale
    xs = (x.reshape(C * K, V) * np.float32(1.0 / GSCALE)).astype(bf16)
    xtd = np.ascontiguousarray(
        xs[:, vm.T].transpose(1, 2, 0).reshape(128, NT * C * K)
    )

    # odd fifo frames 1,3,...,13 -> per-core (28, 1024) bf16 slices
    ff_sh = np.ascontiguousarray(
        fifo.reshape(F, C, NCORES, WS)[1:14:2]
        .transpose(2, 0, 1, 3)
        .reshape(NCORES, 7 * C, WS)
    ).astype(bf16)
    eye = np.eye(K, dtype=np.float32)
    selfm = np.ascontiguousarray(np.tile(eye, (7, 1))).astype(bf16)

    if _CACHED_NC is None:
        _CACHED_NC = _build_nc()
    nc = _CACHED_NC

    in_maps = [
        {
            "g8": g8_sh[m], "g4": g4_sh[m], "xtd": xtd,
            "ffhi": ff_sh[m], "selfm": selfm,
        }
        for m in range(NCORES)
    ]
    res = run_bass_kernel_spmd(
        nc, in_maps, core_ids=list(range(NCORES)), trace=TRACE
    )
    LAST = res
    b = np.concatenate([res.results[m]["out"] for m in range(NCORES)], axis=1)
    return np.ascontiguousarray(b.reshape(1, C, V, 1))


# revision 11
# speedup vs baseline: 1.2217x; 1.2217x over previous
"""Trainium2 Bass kernel for nn_AggregateStgcn (gnn_message_passing).

Computes, for x:(1,16,1,8192) f32, graph:(8192,8192) f32, fifo:(1,16,4,8192) f32,
stride=2:
    Asum[k, v] = sum_c x[0, c*4+k, 0, v]              (4, 8192)
    xsum[k, w] = sum_v Asum[k, v] * graph[v, w]       (4, 8192)
    S[k, w]    = sum_{j in 1,3,...,13} fifo[0, j, k, w]
    out[0, k, w, 0] = xsum[k, w] + S[k, w]            (1, 4, 8192, 1)

Sharding: graph is split column-wise across 8 NeuronCores (tensor parallel over
output nodes w); x is replicated; the fifo slice is local per core. No
collectives; host concatenates the 8 (4, 1024) output slices.

Precision/perf strategy: the kernel streams the whole graph once, so it is
HBM-bandwidth-bound; the graph is quantized on the host to 1 byte/element.
Rows are split in two regimes to balance the PE against the DMA:
 - tiles 0..NSPLIT-1: fp8 E3M4 (4 mantissa bits) at scale 256, matmul'd
   against the bf16 activation (A/256) at 1 moving element/cycle;
 - tiles NSPLIT..63: fp8 E4M3 at scale 32 in DoubleRow perf mode (2 graph
   elements/cycle - HW-verified the per-256-row pair costs one matmul slot).
   DoubleRow needs an fp8 stationary, so the activation rides as E4M3 hi
   (cols 0:4 of the packed stationary, = A/32) plus E4M3 lo of the residual
   x16 (cols 32:36); hi products land at scale 1 directly in the main
   accumulator rows 0:4, lo products land on rows 32:36 and are folded in as
   out += lo_acc/16 - two DVE ops total.
All matmuls accumulate into ONE (48, 1024) PSUM region; the fifo matmul
(selfm zero-padded to 48 columns) opens it, zeroing the lo rows too.
Measured end-to-end error of this scheme on the real inputs: 1.26e-2
(max-err / max-expected) vs the 2e-2 gate, fully deterministic (the HW
matmul paths are exact on quantized values; verified against host sim).

Layout: the host pre-permutes the graph slice into the exact per-partition
stream order (for a chunk of s row-tiles starting at row off*128, partition p
holds rows off*128 + p*s + j, j=0..s-1), so every DMA is a plain 2D slice
with one long contiguous run per partition. A DoubleRow pair contracts
tiles (2d, 2d+1) as k2=0/1 (contraction index = partition + 128*k2), i.e.
two consecutive stream tiles - the moving AP is just a (128, 2, 512) view of
the chunk. x is shipped pre-transposed and identically permuted as
(128, 64*16) bf16 (pre-divided by 256); three DVE adds reduce its 16
channels to the (128, 64, 4) bf16 stationary, and four more small DVE ops
build the packed E4M3 hi/lo stationary for the DoubleRow region.

Schedule: ALL graph chunks are queued up front and every chunk gets its own
SBUF buffer (8.4 MB total - no buffer-ring backpressure, the DMA free-runs
at HBM rate). Tiny chunks first so the first matmuls start early, 512 KB
middles alternating across the two HWDGE queues, DoubleRow chunks last
(they drain 2x faster after the final bytes land). Warmup matmuls on
uninitialized data (results never read) open the PE HAM clock gate from the
first possible cycle, and filler matmuls after the early chunks bridge
arrival gaps so the gate never drops to half clock.
"""

import numpy as np

V = 8192
C = 4
K = 4
F = 16
NCORES = 8
WS = V // NCORES          # 1024 output columns per core
NT = V // 128             # 64 contraction tiles
NSPLIT = 40               # tiles 0..39 e3m4 normal, 40..63 e4m3 DoubleRow
# v-tiles per DMA chunk (normal region, then DoubleRow region)
CHUNKS_N = [1, 1, 2, 2] + [4] * 8 + [2]
CHUNKS_D = [4, 4, 4, 4, 4, 2, 2]
assert sum(CHUNKS_N) == NSPLIT and sum(CHUNKS_D) == NT - NSPLIT
CHUNKS = CHUNKS_N + CHUNKS_D
WARMUP_MM = 10            # throwaway matmuls to open the PE clock gate
FILLER_CHUNKS = 6         # early chunks followed by filler matmuls
GSCALE = 256.0            # e3m4 graph pre-scale (x is pre-divided by this)
G4SCALE = 32.0            # e4m3 graph pre-scale for the DoubleRow region
DRW = 48                  # DoubleRow stationary packed width (16B-aligned)

TRACE = False             # set by test harness to capture an NTFF profile
LAST = None               # BassKernelResults of the most recent run

_CACHED_NC = None


def _offs():
    return np.cumsum([0] + CHUNKS).tolist()


def _vmap():
    """vmap[t, p] = graph row held by partition p for contraction tile t."""
    offs = _offs()
    vm = np.empty((NT, 128), np.int64)
    for ci, s in enumerate(CHUNKS):
        off = offs[ci]
        for j in range(s):
            vm[off + j] = off * 128 + np.arange(128) * s + j
    return vm


def _build_nc():
    import concourse.bacc as bacc
    import concourse.mybir as mybir
    from concourse.tile import TileContext

    f32 = mybir.dt.float32
    bf16 = mybir.dt.bfloat16
    f8e3 = mybir.dt.float8e3
    f8e4 = mybir.dt.float8e4
    nc = bacc.Bacc(
        "TRN2",
        target_bir_lowering=False,
        debug=False,
        enable_asserts=False,
        num_devices=NCORES,
    )
    NDR = NT - NSPLIT
    g8 = nc.dram_tensor("g8", [128, NSPLIT * WS], f8e3, kind="ExternalInput")
    g4 = nc.dram_tensor("g4", [128, NDR * WS], f8e4, kind="ExternalInput")
    xtd = nc.dram_tensor("xtd", [128, NT * C * K], bf16, kind="ExternalInput")
    ffhi = nc.dram_tensor("ffhi", [7 * C, WS], bf16, kind="ExternalInput")
    selfm = nc.dram_tensor("selfm", [7 * C, DRW], bf16, kind="ExternalInput")
    out = nc.dram_tensor("out", [K, WS], f32, kind="ExternalOutput")

    offs = _offs()
    n_chunks = len(CHUNKS)

    with TileContext(nc) as tc:
        with (
            tc.tile_pool(name="const", bufs=1) as cpool,
            tc.tile_pool(name="gp", bufs=1) as gpool,
            tc.tile_pool(name="ps", bufs=1, space="PSUM") as ppool,
        ):
            # PE warmup (outputs never read): the memset rides GPSIMD, whose
            # queue opens ~1.3us before DVE's, so the first warmup matmul
            # issues as early as possible and the HAM clock gate starts
            # warming immediately.
            wtile = cpool.tile([128, 512], bf16)
            nc.gpsimd.memset(wtile[:], 1.0)
            wps = ppool.tile([128, 512], f32)

            def filler():
                nc.tensor.matmul(
                    wps[:], wtile[:, 0:128], wtile[:], start=True, stop=True
                )

            for _ in range(WARMUP_MM):
                filler()

            # small inputs first on both HWDGE queues, ahead of the graph
            xtd_sb = cpool.tile([128, NT * C * K], bf16)
            nc.sync.dma_start(out=xtd_sb[:], in_=xtd.ap())
            selfm_sb = cpool.tile([7 * C, DRW], bf16)
            nc.scalar.dma_start(out=selfm_sb[:], in_=selfm.ap())
            ffhi_sb = cpool.tile([7 * C, WS], bf16)
            nc.scalar.dma_start(out=ffhi_sb[:], in_=ffhi.ap())

            # graph chunk DMAs: every chunk has its own buffer (no ring
            # backpressure) and all transfers are queued up front
            gts = []
            for ci, s in enumerate(CHUNKS):
                off = offs[ci]
                if ci < len(CHUNKS_N):
                    gt = gpool.tile([128, s * WS], f8e3, name="gt", tag=f"gt{ci}")
                    src = g8.ap()[:, off * WS : (off + s) * WS]
                else:
                    doff = off - NSPLIT
                    gt = gpool.tile([128, s * WS], f8e4, name="gt4", tag=f"gt{ci}")
                    src = g4.ap()[:, doff * WS : (doff + s) * WS]
                if ci % 2 == 0:
                    nc.sync.dma_start(out=gt[:], in_=src)
                else:
                    nc.scalar.dma_start(out=gt[:], in_=src)
                gts.append(gt)

            # DVE prep: reduce the 16 channels of the pre-transposed x to the
            # (128, 64, 4) stationary operand (bf16 in/out for 2x DVE rate)
            xv = xtd_sb.rearrange("p (t a) -> p t a", a=C * K)
            t0 = cpool.tile([128, NT, K], bf16)
            nc.vector.tensor_add(out=t0[:], in0=xv[:, :, 0:K], in1=xv[:, :, K : 2 * K])
            t1 = cpool.tile([128, NT, K], bf16)
            nc.vector.tensor_add(
                out=t1[:], in0=xv[:, :, 2 * K : 3 * K], in1=xv[:, :, 3 * K :]
            )
            asum = cpool.tile([128, NT, K], bf16)
            nc.vector.tensor_add(out=asum[:], in0=t0[:], in1=t1[:])

            # DoubleRow stationary: hi = e4m3(asum*8) = A/32 in cols 0:4,
            # lo = e4m3((asum*8 - hi)*16) in cols 32:36; other columns are
            # junk (their product rows are never read)
            asum_dr = cpool.tile([128, NDR, DRW], f8e4)
            a8 = cpool.tile([128, NDR, K], f32)
            nc.vector.tensor_scalar_mul(a8[:], asum[:, NSPLIT:, :], 8.0)
            nc.vector.tensor_copy(out=asum_dr[:, :, 0:K], in_=a8[:])
            resid = cpool.tile([128, NDR, K], f32)
            nc.vector.tensor_sub(out=resid[:], in0=a8[:], in1=asum_dr[:, :, 0:K])
            nc.vector.tensor_scalar_mul(asum_dr[:, :, 32:36], resid[:], 16.0)

            # single accumulator for everything: rows 0:4 = fifo + normal +
            # DR-hi sums (all at scale 1), rows 32:36 = DR-lo sums (16x).
            # The fifo matmul (selfm zero-padded to 48 cols) opens the group
            # and zeroes all 48 rows.
            accT = ppool.tile([DRW, WS], f32, name="acc", tag="acc")
            acc = [accT[:, 0:512], accT[:, 512:1024]]
            for h in range(2):
                hs = slice(h * 512, (h + 1) * 512)
                nc.tensor.matmul(
                    acc[h], selfm_sb[:], ffhi_sb[:, hs], start=True, stop=False
                )

            for ci, s in enumerate(CHUNKS_N):
                off = offs[ci]
                gt = gts[ci]
                for j in range(s):
                    t = off + j
                    lhsT = asum[:, t, :]
                    for h in range(2):
                        hs = slice(j * WS + h * 512, j * WS + (h + 1) * 512)
                        nc.tensor.matmul(
                            acc[h][0:K, :], lhsT, gt[:, hs],
                            start=False, stop=False,
                        )
                if 1 <= ci <= FILLER_CHUNKS:
                    # bridge early arrival gaps so the HAM clock gate never
                    # sees an idle window and drops the PE to half clock
                    filler()
                    filler()

            dr = mybir.MatmulPerfMode.DoubleRow
            for ci in range(len(CHUNKS_N), n_chunks):
                s = CHUNKS[ci]
                off = offs[ci]
                gtv = gts[ci].rearrange("p (j w) -> p j w", w=WS)
                for d in range(s // 2):
                    gdt = off - NSPLIT + 2 * d
                    lastd = gdt == NDR - 2
                    lhsT = asum_dr[:, gdt : gdt + 2, :]
                    for h in range(2):
                        nc.tensor.matmul(
                            accT[:, h * 512 : (h + 1) * 512],
                            lhsT,
                            gtv[:, 2 * d : 2 * d + 2, h * 512 : (h + 1) * 512],
                            start=False,
                            stop=lastd,
                            perf_mode=dr,
                        )

            # fold lo rows and ship: out = acc[0:4] + acc[32:36]/16
            lo_sb = cpool.tile([K, WS], f32)
            nc.vector.tensor_scalar_mul(lo_sb[:], accT[32:36, :], 1.0 / 16.0)
            out_sb = cpool.tile([K, WS], f32)
            nc.vector.tensor_add(out=out_sb[:], in0=lo_sb[:], in1=accT[0:K, :])
            nc.sync.dma_start(out=out.ap(), in_=out_sb[:])

    nc.compile()
    return nc


def kernel(x, graph, fifo, stride):
    global _CACHED_NC, LAST
    import ml_dtypes
    from concourse.bass_utils import run_bass_kernel_spmd

    bf16 = ml_dtypes.bfloat16
    e3m4 = ml_dtypes.float8_e3m4
    e4m3 = ml_dtypes.float8_e4m3
    x = np.asarray(x, dtype=np.float32)
    graph = np.asarray(graph, dtype=np.float32)
    fifo = np.asarray(fifo, dtype=np.float32)
    stride_v = int(np.asarray(stride))
    assert stride_v == 2, f"kernel hardcodes stride=2, got {stride_v}"

    vm = _vmap()                                  # (NT, 128)

    # graph rows permuted into stream order; e3m4 region then e4m3 region
    rows_n = np.ascontiguousarray(vm[:NSPLIT].T).reshape(-1)
    rows_d = np.ascontiguousarray(vm[NSPLIT:].T).reshape(-1)
    gq3 = np.clip(graph[rows_n] * GSCALE, -15.5, 15.5).astype(e3m4)
    gq4 = np.clip(graph[rows_d] * G4SCALE, -240.0, 240.0).astype(e4m3)
    g3v = gq3.reshape(128, NSPLIT, NCORES, WS)
    g4v = gq4.reshape(128, NT - NSPLIT, NCORES, WS)
    g8_sh = [
        np.ascontiguousarray(g3v[:, :, m]).reshape(128, NSPLIT * WS)
        for m in range(NCORES)
    ]
    g4_sh = [
        np.ascontiguousarray(g4v[:, :, m]).reshape(128, (NT - NSPLIT) * WS)
        for m in range(NCORES)
    ]

    # x -> (128, NT*16) bf16, transposed + identically permuted, pre-divided
    # by the e3m4 graph scale
    xs = (x.reshape(C * K, V) * np.float32(1.0 / GSCALE)).astype(bf16)
    xtd = np.ascontiguousarray(
        xs[:, vm.T].transpose(1, 2, 0).reshape(128, NT * C * K)
    )

    # odd fifo frames 1,3,...,13 -> per-core (28, 1024) bf16 slices
    ff_sh = np.ascontiguousarray(
        fifo.reshape(F, C, NCORES, WS)[1:14:2]
        .transpose(2, 0, 1, 3)
        .reshape(NCORES, 7 * C, WS)
    ).astype(bf16)
    eye = np.eye(K, dtype=np.float32)
    selfm = np.zeros((7 * C, DRW), np.float32)
    selfm[:, 0:K] = np.tile(eye, (7, 1))
    selfm = np.ascontiguousarray(selfm).astype(bf16)

    if _CACHED_NC is None:
        _CACHED_NC = _build_nc()
    nc = _CACHED_NC

    in_maps = [
        {
            "g8": g8_sh[m], "g4": g4_sh[m], "xtd": xtd,
            "ffhi": ff_sh[m], "selfm": selfm,
        }
        for m in range(NCORES)
    ]
    res = run_bass_kernel_spmd(
        nc, in_maps, core_ids=list(range(NCORES)), trace=TRACE
    )
    LAST = res
    b = np.concatenate([res.results[m]["out"] for m in range(NCORES)], axis=1)
    return np.ascontiguousarray(b.reshape(1, C, V, 1))


# revision 13
# speedup vs baseline: 1.3605x; 1.1137x over previous
"""Trainium2 Bass kernel for nn_AggregateStgcn (gnn_message_passing).

Computes, for x:(1,16,1,8192) f32, graph:(8192,8192) f32, fifo:(1,16,4,8192) f32,
stride=2:
    Asum[k, v] = sum_c x[0, c*4+k, 0, v]              (4, 8192)
    xsum[k, w] = sum_v Asum[k, v] * graph[v, w]       (4, 8192)
    S[k, w]    = sum_{j in 1,3,...,13} fifo[0, j, k, w]
    out[0, k, w, 0] = xsum[k, w] + S[k, w]            (1, 4, 8192, 1)

Sharding: graph is split column-wise across 8 NeuronCores (tensor parallel over
output nodes w); x is replicated; the fifo slice is local per core. No
collectives; host concatenates the 8 (4, 1024) output slices.

Precision/perf strategy: the kernel streams the whole graph once (1 byte per
element), and the PE streams every moving element once, so runtime is
max(HBM, PE-stream) plus ramp/tail overheads. Graph rows are split:
 - "normal" tiles: fp8 E3M4 (4 mantissa bits) at scale 256 against the bf16
   activation (A/256): 1 graph element/PE-cycle;
 - DR tiles (24 of 64, the middle of the stream): fp8 E4M3 at scale 32 in
   DoubleRow perf mode: 2 graph elements/PE-cycle (HW-verified 216 ns per
   256-row x 512-col pair - same slot cost as a normal 128-row matmul).
   DoubleRow needs an fp8 stationary, so the activation rides as E4M3 hi
   (cols 0:4, = A/32, products at scale 1) plus E4M3 lo of the residual x16
   (cols 32:36, products 16x). DR accumulates in its own (48, 1024) PSUM
   region that closes early (mid-stream), so the lo fold
   (dr_part = hi_acc + lo_acc/16, two ~1.2us DVE ops) hides under the tail
   matmuls; the final tail is one DVE add + the output DMA.
Measured end-to-end error on the real inputs: 1.26e-2 (max-err/max-expected)
vs the 2e-2 gate, deterministic (HW matmul is exact on the quantized values;
verified against host simulation).

Layout: the host pre-permutes the graph slice into the exact per-partition
stream order (for a chunk of s row-tiles starting at row off*128, partition p
holds rows off*128 + p*s + j), so every DMA is a plain 2D slice with one
long contiguous run per partition. A DoubleRow pair contracts two
consecutive stream tiles as k2=0/1 (contraction index = partition + 128*k2),
i.e. the moving AP is a (128, 2, 512) view of the chunk. x is shipped
pre-transposed and identically permuted as (128, 64*16) bf16 (pre-divided by
256); three DVE adds reduce its 16 channels to the (128, 64, 4) bf16
stationary, four more small DVE ops build the packed E4M3 hi/lo stationary.

Schedule: ALL graph chunks are queued up front, each with its own SBUF
buffer (8.4 MB resident - no buffer-ring backpressure, DMA free-runs at
~360 GB/s). Tiny chunks first so the first matmuls start early; the
DoubleRow chunks ride in the middle; 512 KB chunks alternate across the two
HWDGE queues. Warmup matmuls (memset on GPSIMD, whose queue opens first)
open the PE HAM clock gate immediately; filler matmuls bridge the first
chunk-arrival gaps so the gate never drops to half clock.
"""

import numpy as np

V = 8192
C = 4
K = 4
F = 16
NCORES = 8
WS = V // NCORES          # 1024 output columns per core
NT = V // 128             # 64 contraction tiles
# chunk schedule: (kind, tiles); DR region mid-stream, 24 tiles
CHUNK_PLAN = (
    [("h", 1), ("h", 1), ("h", 2), ("h", 2)]
    + [("d", 4)] * 5 + [("d", 2)] * 2
    + [("t", 4)] * 8 + [("t", 2)]
)
CHUNKS = [s for _, s in CHUNK_PLAN]
NHEAD = sum(s for k, s in CHUNK_PLAN if k == "h")    # 6  e3m4 tiles
NDR = sum(s for k, s in CHUNK_PLAN if k == "d")      # 24 e4m3 DR tiles
NTAIL = sum(s for k, s in CHUNK_PLAN if k == "t")    # 34 e3m4 tiles
assert NHEAD + NDR + NTAIL == NT
DR_LO = NHEAD                # first DR tile index in stream order
DR_HI = NHEAD + NDR          # one past last
WARMUP_MM = 6                # throwaway matmuls to open the PE clock gate
GSCALE = 256.0               # e3m4 graph pre-scale (x is pre-divided by this)
G4SCALE = 32.0               # e4m3 graph pre-scale for the DoubleRow region
DRW = 48                     # DoubleRow stationary packed width (16B-aligned)

TRACE = False                # set by test harness to capture an NTFF profile
LAST = None                  # BassKernelResults of the most recent run

_CACHED_NC = None


def _offs():
    return np.cumsum([0] + CHUNKS).tolist()


def _vmap():
    """vmap[t, p] = graph row held by partition p for contraction tile t."""
    offs = _offs()
    vm = np.empty((NT, 128), np.int64)
    for ci, s in enumerate(CHUNKS):
        off = offs[ci]
        for j in range(s):
            vm[off + j] = off * 128 + np.arange(128) * s + j
    return vm


def _build_nc():
    import concourse.bacc as bacc
    import concourse.mybir as mybir
    from concourse.tile import TileContext

    f32 = mybir.dt.float32
    bf16 = mybir.dt.bfloat16
    f8e3 = mybir.dt.float8e3
    f8e4 = mybir.dt.float8e4
    nc = bacc.Bacc(
        "TRN2",
        target_bir_lowering=False,
        debug=False,
        enable_asserts=False,
        num_devices=NCORES,
    )
    gh = nc.dram_tensor("gh", [128, NHEAD * WS], f8e3, kind="ExternalInput")
    gd = nc.dram_tensor("gd", [128, NDR * WS], f8e4, kind="ExternalInput")
    gt2 = nc.dram_tensor("gt2", [128, NTAIL * WS], f8e3, kind="ExternalInput")
    xtd = nc.dram_tensor("xtd", [128, NT * C * K], bf16, kind="ExternalInput")
    ffhi = nc.dram_tensor("ffhi", [7 * C, WS], bf16, kind="ExternalInput")
    selfm = nc.dram_tensor("selfm", [7 * C, K], bf16, kind="ExternalInput")
    out = nc.dram_tensor("out", [K, WS], f32, kind="ExternalOutput")

    offs = _offs()

    with TileContext(nc) as tc:
        with (
            tc.tile_pool(name="const", bufs=1) as cpool,
            tc.tile_pool(name="gp", bufs=1) as gpool,
            tc.tile_pool(name="ps", bufs=1, space="PSUM") as ppool,
        ):
            # PE warmup (outputs never read): the memset rides GPSIMD, whose
            # queue opens ~1.3us before DVE's, so the first warmup matmul
            # issues as early as possible and the HAM gate warms immediately.
            wtile = cpool.tile([128, 512], bf16)
            nc.gpsimd.memset(wtile[:], 1.0)
            wps = ppool.tile([128, 512], f32)

            def filler():
                nc.tensor.matmul(
                    wps[:], wtile[:, 0:128], wtile[:], start=True, stop=True
                )

            for _ in range(WARMUP_MM):
                filler()

            # small inputs first on both HWDGE queues, ahead of the graph
            xtd_sb = cpool.tile([128, NT * C * K], bf16)
            nc.sync.dma_start(out=xtd_sb[:], in_=xtd.ap())
            selfm_sb = cpool.tile([7 * C, K], bf16)
            nc.scalar.dma_start(out=selfm_sb[:], in_=selfm.ap())
            ffhi_sb = cpool.tile([7 * C, WS], bf16)
            nc.scalar.dma_start(out=ffhi_sb[:], in_=ffhi.ap())

            # graph chunk DMAs: every chunk has its own resident buffer and
            # all transfers are queued up front (no ring backpressure)
            gts = []
            for ci, (kind, s) in enumerate(CHUNK_PLAN):
                off = offs[ci]
                if kind == "h":
                    src = gh.ap()[:, off * WS : (off + s) * WS]
                    dt_ = f8e3
                elif kind == "d":
                    doff = off - DR_LO
                    src = gd.ap()[:, doff * WS : (doff + s) * WS]
                    dt_ = f8e4
                else:
                    toff = off - DR_HI
                    src = gt2.ap()[:, toff * WS : (toff + s) * WS]
                    dt_ = f8e3
                gt = gpool.tile([128, s * WS], dt_, name="gt", tag=f"gt{ci}")
                if ci % 2 == 0:
                    nc.sync.dma_start(out=gt[:], in_=src)
                else:
                    nc.scalar.dma_start(out=gt[:], in_=src)
                gts.append(gt)

            # DVE prep: reduce the 16 channels of the pre-transposed x to the
            # (128, 64, 4) stationary operand (bf16 in/out for 2x DVE rate)
            xv = xtd_sb.rearrange("p (t a) -> p t a", a=C * K)
            t0 = cpool.tile([128, NT, K], bf16)
            nc.vector.tensor_add(out=t0[:], in0=xv[:, :, 0:K], in1=xv[:, :, K : 2 * K])
            t1 = cpool.tile([128, NT, K], bf16)
            nc.vector.tensor_add(
                out=t1[:], in0=xv[:, :, 2 * K : 3 * K], in1=xv[:, :, 3 * K :]
            )
            asum = cpool.tile([128, NT, K], bf16)
            nc.vector.tensor_add(out=asum[:], in0=t0[:], in1=t1[:])

            # DoubleRow stationary: hi = e4m3(asum*8) = A/32 in cols 0:4,
            # lo = e4m3((asum*8 - hi)*16) in cols 32:36; other columns are
            # junk (their product rows are never read)
            asum_dr = cpool.tile([128, NDR, DRW], f8e4)
            a8 = cpool.tile([128, NDR, K], f32)
            nc.vector.tensor_scalar_mul(a8[:], asum[:, DR_LO:DR_HI, :], 8.0)
            nc.vector.tensor_copy(out=asum_dr[:, :, 0:K], in_=a8[:])
            resid = cpool.tile([128, NDR, K], f32)
            nc.vector.tensor_sub(out=resid[:], in0=a8[:], in1=asum_dr[:, :, 0:K])
            nc.vector.tensor_scalar_mul(asum_dr[:, :, 32:36], resid[:], 16.0)

            # accumulators: (4, 1024) main (fifo + normal tiles, scale 1) and
            # (48, 1024) DR (rows 0:4 hi at scale 1, rows 32:36 lo at 16x)
            accM = ppool.tile([K, WS], f32, name="accM", tag="accM")
            accD = ppool.tile([DRW, WS], f32, name="accD", tag="accD")
            for h in range(2):
                hs = slice(h * 512, (h + 1) * 512)
                nc.tensor.matmul(
                    accM[:, hs], selfm_sb[:], ffhi_sb[:, hs],
                    start=True, stop=False,
                )

            def normal_chunk(ci, last_ci):
                s = CHUNKS[ci]
                off = offs[ci]
                gt = gts[ci]
                for j in range(s):
                    t = off + j
                    lastt = last_ci and j == s - 1
                    lhsT = asum[:, t, :]
                    for h in range(2):
                        hs = slice(j * WS + h * 512, j * WS + (h + 1) * 512)
                        nc.tensor.matmul(
                            accM[:, h * 512 : (h + 1) * 512],
                            lhsT, gt[:, hs], start=False, stop=lastt,
                        )

            dr = mybir.MatmulPerfMode.DoubleRow
            n_head_chunks = sum(1 for k, _ in CHUNK_PLAN if k == "h")
            n_dr_chunks = sum(1 for k, _ in CHUNK_PLAN if k == "d")

            # head: tiny e3m4 chunks with fillers bridging early DMA gaps
            for ci in range(n_head_chunks):
                normal_chunk(ci, False)
                if ci >= 1:
                    filler()
                    filler()

            # middle: DoubleRow chunks into accD (start on the first pair,
            # stop on the last - accD is finalized mid-stream)
            for ci in range(n_head_chunks, n_head_chunks + n_dr_chunks):
                s = CHUNKS[ci]
                off = offs[ci]
                gtv = gts[ci].rearrange("p (j w) -> p j w", w=WS)
                for d in range(s // 2):
                    gdt = off - DR_LO + 2 * d
                    lhsT = asum_dr[:, gdt : gdt + 2, :]
                    for h in range(2):
                        nc.tensor.matmul(
                            accD[:, h * 512 : (h + 1) * 512],
                            lhsT,
                            gtv[:, 2 * d : 2 * d + 2, h * 512 : (h + 1) * 512],
                            start=(gdt == 0),
                            stop=(gdt == NDR - 2),
                            perf_mode=dr,
                        )

            # fold the DR region while the tail matmuls run:
            # dr_part = accD[0:4] + accD[32:36]/16
            lo_sb = cpool.tile([K, WS], f32)
            nc.vector.tensor_scalar_mul(lo_sb[:], accD[32:36, :], 1.0 / 16.0)
            dr_part = cpool.tile([K, WS], f32)
            nc.vector.tensor_add(out=dr_part[:], in0=lo_sb[:], in1=accD[0:K, :])

            # tail: e3m4 chunks; the last matmul closes the main group
            for ci in range(n_head_chunks + n_dr_chunks, len(CHUNK_PLAN)):
                normal_chunk(ci, ci == len(CHUNK_PLAN) - 1)

            # final: out = dr_part + accM, then ship
            out_sb = cpool.tile([K, WS], f32)
            nc.vector.tensor_add(out=out_sb[:], in0=dr_part[:], in1=accM[:])
            nc.sync.dma_start(out=out.ap(), in_=out_sb[:])

    nc.compile()
    return nc


def kernel(x, graph, fifo, stride):
    global _CACHED_NC, LAST
    import ml_dtypes
    from concourse.bass_utils import run_bass_kernel_spmd

    bf16 = ml_dtypes.bfloat16
    e3m4 = ml_dtypes.float8_e3m4
    e4m3 = ml_dtypes.float8_e4m3
    x = np.asarray(x, dtype=np.float32)
    graph = np.asarray(graph, dtype=np.float32)
    fifo = np.asarray(fifo, dtype=np.float32)
    stride_v = int(np.asarray(stride))
    assert stride_v == 2, f"kernel hardcodes stride=2, got {stride_v}"

    vm = _vmap()                                  # (NT, 128)

    # graph rows permuted into stream order, per region
    rows_h = np.ascontiguousarray(vm[:DR_LO].T).reshape(-1)
    rows_d = np.ascontiguousarray(vm[DR_LO:DR_HI].T).reshape(-1)
    rows_t = np.ascontiguousarray(vm[DR_HI:].T).reshape(-1)
    qh = np.clip(graph[rows_h] * GSCALE, -15.5, 15.5).astype(e3m4)
    qd = np.clip(graph[rows_d] * G4SCALE, -240.0, 240.0).astype(e4m3)
    qt = np.clip(graph[rows_t] * GSCALE, -15.5, 15.5).astype(e3m4)

    def shard(q, ntile):
        qv = q.reshape(128, ntile, NCORES, WS)
        return [
            np.ascontiguousarray(qv[:, :, m]).reshape(128, ntile * WS)
            for m in range(NCORES)
        ]

    gh_sh = shard(qh, NHEAD)
    gd_sh = shard(qd, NDR)
    gt_sh = shard(qt, NTAIL)

    # x -> (128, NT*16) bf16, transposed + identically permuted, pre-divided
    # by the e3m4 graph scale
    xs = (x.reshape(C * K, V) * np.float32(1.0 / GSCALE)).astype(bf16)
    xtd = np.ascontiguousarray(
        xs[:, vm.T].transpose(1, 2, 0).reshape(128, NT * C * K)
    )

    # odd fifo frames 1,3,...,13 -> per-core (28, 1024) bf16 slices
    ff_sh = np.ascontiguousarray(
        fifo.reshape(F, C, NCORES, WS)[1:14:2]
        .transpose(2, 0, 1, 3)
        .reshape(NCORES, 7 * C, WS)
    ).astype(bf16)
    eye = np.eye(K, dtype=np.float32)
    selfm = np.ascontiguousarray(np.tile(eye, (7, 1))).astype(bf16)

    if _CACHED_NC is None:
        _CACHED_NC = _build_nc()
    nc = _CACHED_NC

    in_maps = [
        {
            "gh": gh_sh[m], "gd": gd_sh[m], "gt2": gt_sh[m], "xtd": xtd,
            "ffhi": ff_sh[m], "selfm": selfm,
        }
        for m in range(NCORES)
    ]
    res = run_bass_kernel_spmd(
        nc, in_maps, core_ids=list(range(NCORES)), trace=TRACE
    )
    LAST = res
    b = np.concatenate([res.results[m]["out"] for m in range(NCORES)], axis=1)
    return np.ascontiguousarray(b.reshape(1, C, V, 1))
